# revision 9
# baseline (speedup 1.0000x reference)
"""Trainium2 Bass kernel for nn_CP_L3_sparse_outer (v3, bf16).

Math (per batch row b):
    s2[b] = sum_d U2[d] * z[b, d]
    s3[b] = sum_d U3[d] * z[b, d]
    out[b, o] = (s2[b] * s3[b]) * sum_d (U1[d] * z[b, d]) * W[o, d] + bias[o]

Sharding: data-parallel over batch B=8192 across 8 NeuronCores
(B_loc = 1024 rows per core); W / U1 / U2 / U3 / bias replicated.

All-bf16 pipeline (measured rel-err 0.29% vs the 2e-2 gate), main matmul
output-natural so no output transposes:

  A. z bf16 [128 rows, 4096] per tile; PE transposes (bf16 = 1 cyc/row)
     in 4-chunk groups through one PSUM bank; ACT copies into resident
     ztbig = z.T [128 d, k(32) * 1024 b], raw.
  B. s2/s3 on PE from raw zT: psum[64, 512] += u23pad.T @ zt (U2 in
     stationary col 0, U3 in col 32 so evictions hit 32-aligned psum
     partitions), 2 batch halves x 32 k.
  C. c = s2*s3 (DVE, in place) -> 8 one-column micro-matmuls -> ccol
     [128 b, 8 bt] so c is a per-partition scalar at eviction.
  D. ztbig *= U1 in place on DVE (u1 lives on partitions), per (k, half).
  E. For each o-chunk (8 x 512): stream wt slab [128 d, 32 k, 512 o]
     (double-buffered SWDGE); per bt: psum[128 b, 512 o] += zt[k, bt]
     (stationary) @ wt[k, oc] (moving) over 32 k; evict with one DVE
     scalar_tensor_tensor: (psum * ccol) + biasb; batched out DMA per oc
     (split in half for the last chunk to shorten the drain tail).

bias[o] is on the free dim at eviction, so it is broadcast across
partitions once via ones-outer-product matmuls into biasb (emitted after
C so the first PE instruction needs no DMA at all). u1/u23 are passed
from the host pre-tiled as [128, 32(,2)] so their one-shot HWDGE loads
are partition-contiguous (the naive strided load cost ~24us of dead time
before the first PE instruction in v2).
Host prep is dtype/layout only: bf16 casts + transposes/reshapes.
"""

import os
import sys

import numpy as np

if "/opt/trn_rl_repo" not in sys.path:
    sys.path.insert(0, "/opt/trn_rl_repo")

import concourse.bass as bass
from concourse import bacc
import concourse.mybir as mybir
import concourse.tile as tile
from concourse.masks import make_identity

P = 128
D = 4096
O = 4096
B = 8192
NCORES = 8
BLOC = B // NCORES          # 1024 batch rows per core
KC = D // P                 # 32 contraction chunks
BT = BLOC // P              # 8 batch tiles of 128
OC = O // 512               # 8 output chunks of 512
NH = BLOC // 512            # 2 batch halves
F32 = mybir.dt.float32
BF16 = mybir.dt.bfloat16
MULT = mybir.AluOpType.mult
ADD = mybir.AluOpType.add
COPY = mybir.ActivationFunctionType.Copy


def build_nc() -> bass.Bass:
    nc = bacc.Bacc(trn_type="TRN2")

    z_d = nc.dram_tensor("z", [BLOC, D], BF16, kind="ExternalInput")
    wt_d = nc.dram_tensor("wt", [D, O], BF16, kind="ExternalInput")
    u1_d = nc.dram_tensor("u1", [P, KC], F32, kind="ExternalInput")
    u23_d = nc.dram_tensor("u23", [P, KC, 2], BF16, kind="ExternalInput")
    bias_d = nc.dram_tensor("bias", [O], BF16, kind="ExternalInput")
    out_d = nc.dram_tensor("out", [BLOC, O], F32, kind="ExternalOutput")

    zview = z_d[:].rearrange("(t p) d -> p t d", p=P)          # [128, 8, 4096]
    wview = wt_d[:].rearrange("(k p) o -> p k o", p=P)         # [128, 32, 4096]
    oview = out_d[:].rearrange("(t p) o -> p t o", p=P)        # [128, 8, 4096]

    with tile.TileContext(nc) as tc:
        with (
            tc.tile_pool(name="const", bufs=1) as const,
            tc.tile_pool(name="ztp", bufs=1) as ztp,
            tc.tile_pool(name="znat", bufs=2) as znatp,
            tc.tile_pool(name="wslab", bufs=2) as wslabp,
            tc.tile_pool(name="onat", bufs=2) as onatp,
            tc.tile_pool(name="pmain", bufs=4, space="PSUM") as pmain,
            tc.tile_pool(name="ptr", bufs=2, space="PSUM") as ptr,
            tc.tile_pool(name="pmisc", bufs=2, space="PSUM") as pmisc,
        ):
            # ---- constants ----
            identity = const.tile([P, P], F32)
            make_identity(nc, identity)
            identity_b = const.tile([P, P], BF16)
            nc.vector.tensor_copy(identity_b[:], identity[:])
            ones1 = const.tile([1, P], BF16)
            nc.vector.memset(ones1[:], 1.0)
            onef = const.tile([1, 1], F32)
            nc.vector.memset(onef[:], 1.0)
            u1sb = const.tile([P, KC], F32)
            nc.sync.dma_start(u1sb[:], u1_d[:])
            u23sb = const.tile([P, KC, 2], BF16)
            nc.sync.dma_start(u23sb[:], u23_d[:])
            # s2/s3 psum rows must land on 32-aligned partitions: put U2 in
            # stationary column 0 and U3 in column 32 of a zero-padded lhsT.
            u23pad = const.tile([P, KC, 64], BF16)
            nc.vector.memset(u23pad[:], 0.0)
            nc.vector.tensor_copy(u23pad[:, :, 0:1], u23sb[:, :, 0:1])
            nc.vector.tensor_copy(u23pad[:, :, 32:33], u23sb[:, :, 1:2])
            biasrow = znatp.tile([1, O], BF16, name="znat")
            nc.sync.dma_start(biasrow[:], bias_d[:].rearrange("(a o) -> a o", a=1))
            biasb = const.tile([P, O], BF16)
            t2row = const.tile([1, BLOC], F32)
            t3row = const.tile([1, BLOC], F32)
            ccol = const.tile([P, BT], F32)

            # warm-up transpose: first PE instruction, depends on no DMA
            ptw = ptr.tile([P, 512], BF16, name="pt", tag="pt")
            nc.tensor.transpose(ptw[:, 0:P], identity_b[:], identity_b[:])

            # zT resident: [128 d_in, k * BLOC + b]
            ztbig = ztp.tile([P, KC * BLOC], BF16)
            zt3 = ztbig[:].rearrange("p (k r) -> p k r", r=BLOC)

            # ---- phase A: load + transpose z ----
            for bt in range(BT):
                znat = znatp.tile([P, D], BF16, name="znat")
                nc.gpsimd.dma_start(znat[:], zview[:, bt, :])
                for g in range(KC // 4):
                    pt = ptr.tile([P, 512], BF16, name="pt", tag="pt")
                    for i in range(4):
                        nc.tensor.transpose(
                            pt[:, i * P : (i + 1) * P],
                            znat[:, (g * 4 + i) * P : (g * 4 + i + 1) * P],
                            identity_b[:],
                        )
                    nc.scalar.activation(
                        zt3[:, g * 4 : g * 4 + 4, bt * P : (bt + 1) * P],
                        pt[:].rearrange("p (k r) -> p k r", r=P),
                        COPY,
                    )

            # ---- phase B: s2/s3 from RAW zt, wide moving operand ----
            for h in range(NH):
                ps23 = pmisc.tile([64, 512], F32, name="ps23", tag="pmisc")
                for k in range(KC):
                    nc.tensor.matmul(
                        ps23[:],
                        u23pad[:, k, :],
                        zt3[:, k, h * 512 : (h + 1) * 512],
                        start=(k == 0),
                        stop=(k == KC - 1),
                    )
                nc.vector.tensor_copy(
                    t2row[0:1, h * 512 : (h + 1) * 512], ps23[0:1, :]
                )
                nc.vector.tensor_copy(
                    t3row[0:1, h * 512 : (h + 1) * 512], ps23[32:33, :]
                )

            # ---- phase C: c = s2*s3 (in place) -> ccol [128 b, bt] ----
            nc.vector.tensor_mul(t2row[0:1, :], t2row[0:1, :], t3row[0:1, :])
            pc = pmisc.tile([P, BT], F32, name="pc", tag="pmisc")
            for g in range(BT):
                nc.tensor.matmul(
                    pc[:, g : g + 1],
                    t2row[0:1, g * P : (g + 1) * P],
                    onef[0:1, 0:1],
                    start=True, stop=True,
                )
            nc.vector.tensor_copy(ccol[:], pc[:])

            # ---- phase D: fold U1 into zt in place (per-partition scalar) ----
            for h in range(NH):
                for k in range(KC):
                    nc.vector.tensor_scalar_mul(
                        zt3[:, k, h * 512 : (h + 1) * 512],
                        zt3[:, k, h * 512 : (h + 1) * 512],
                        u1sb[:, k : k + 1],
                    )

            # bias broadcast across partitions: biasb[p, o] = bias[o]
            for oc in range(OC):
                pb = pmisc.tile([P, 512], F32, name="pb", tag="pmisc")
                nc.tensor.matmul(
                    pb[:], ones1[:], biasrow[0:1, oc * 512 : (oc + 1) * 512],
                    start=True, stop=True,
                )
                nc.scalar.activation(biasb[:, oc * 512 : (oc + 1) * 512], pb[:], COPY)

            # ---- phase E: main matmul, output-natural psum [b, o] ----
            for oc in range(OC):
                ws = wslabp.tile([P, KC, 512], BF16, name="wslab")
                nc.gpsimd.dma_start(ws[:], wview[:, :, oc * 512 : (oc + 1) * 512])
                onat = onatp.tile([P, BT, 512], F32, name="onat")
                for bt in range(BT):
                    pm = pmain.tile([P, 512], F32, name="pm", tag="pmain")
                    for k in range(KC):
                        nc.tensor.matmul(
                            pm[:],
                            zt3[:, k, bt * P : (bt + 1) * P],
                            ws[:, k, :],
                            start=(k == 0),
                            stop=(k == KC - 1),
                        )
                    nc.vector.scalar_tensor_tensor(
                        onat[:, bt, :],
                        pm[:],
                        ccol[:, bt : bt + 1],
                        biasb[:, oc * 512 : (oc + 1) * 512],
                        MULT,
                        ADD,
                    )
                if oc == OC - 1:
                    # split the last store so the drain tail is half as long
                    nc.gpsimd.dma_start(
                        oview[:, 0 : BT // 2, oc * 512 : (oc + 1) * 512],
                        onat[:, 0 : BT // 2, :],
                    )
                    nc.gpsimd.dma_start(
                        oview[:, BT // 2 : BT, oc * 512 : (oc + 1) * 512],
                        onat[:, BT // 2 : BT, :],
                    )
                else:
                    nc.gpsimd.dma_start(
                        oview[:, :, oc * 512 : (oc + 1) * 512], onat[:]
                    )

    nc.finalize()
    return nc


_NC_CACHE = {}


def get_nc() -> bass.Bass:
    if "nc" not in _NC_CACHE:
        _NC_CACHE["nc"] = build_nc()
    return _NC_CACHE["nc"]


def kernel(z, U1, U2, U3, W, b):
    import ml_dtypes
    from concourse.bass_utils import run_bass_kernel_spmd

    bf = ml_dtypes.bfloat16
    z = np.ascontiguousarray(np.asarray(z, dtype=np.float32)).reshape(B, D)
    zq = z.astype(bf)
    u1t = np.ascontiguousarray(
        np.asarray(U1, dtype=np.float32).reshape(KC, P).T
    )
    wt = np.ascontiguousarray(np.asarray(W, dtype=np.float32).T).astype(bf)
    u23 = np.stack(
        [np.asarray(U2, dtype=np.float32), np.asarray(U3, dtype=np.float32)], 1
    )
    u23t = np.ascontiguousarray(
        u23.reshape(KC, P, 2).transpose(1, 0, 2)
    ).astype(bf)
    bias = np.asarray(b, dtype=np.float32).astype(bf)

    nc = get_nc()
    in_maps = [
        {
            "z": zq[c * BLOC : (c + 1) * BLOC],
            "wt": wt,
            "u1": u1t,
            "u23": u23t,
            "bias": bias,
        }
        for c in range(NCORES)
    ]
    res = run_bass_kernel_spmd(
        nc,
        in_maps,
        core_ids=list(range(NCORES)),
        trace=bool(int(os.environ.get("KERNEL_TRACE", "0"))),
    )
    if res.exec_time_ns is not None:
        print(f"HW exec time: {res.exec_time_ns} ns", file=sys.stderr)
    kernel.last_results = res
    return np.concatenate([res.results[c]["out"] for c in range(NCORES)], axis=0)
